# revision 73
# baseline (speedup 1.0000x reference)
"""GRU-ODE (Neural ODE, dopri5 reference) Trainium2 kernel — fp8 final.

Contract: kernel(**inputs) takes FULL inputs (x0 [1024,1024], t [16],
W_hr/W_hz/W_hh [1024,1024], all fp32) and returns the FULL output
[1024, 16, 1024] fp32 approximating
    odeint(f, x0, t, rtol=1e-5, atol=1e-6)  (dopri5)  transposed to [B,T,H]
with f(h) = (1-sigmoid(h@Wz.T)) * (tanh((sigmoid(h@Wr.T)*h)@Wh.T) - h).

Scheme: data-parallel over batch (128 rows/core). ONE RK4 step across the
whole span with k4 reused as the end derivative (4 f-evals), cubic
Hermite dense output. Numpy-validated (proto.py) rel err 8.0e-3; HW
8.37e-3 vs the 2e-2 gate. HW exec 81.1-84.8us across device clock
draws (from 91.2us bf16 baseline). Late trims: j13-15 drop their
tiny-coefficient m1 term; j9 is a 2-STT DVE preview emitted mid-tail
(after j8) so its chain never delays the final DMA.

Design (all HW-measured; see work/trace analysis in the transcript):
- Gate matmuls fp8-e4m3 DoubleRow: stationary [128,2,128] packed weight
  k-tile pairs, moving = state quarter [128,2,128], contraction 256 per
  instr at ~68-78ns vs bf16's 62.6ns for half the work => ~1.7-1.85x.
  ml_dtypes.float8_e4m3 matches the HW byte format (verified vs a
  host-model DoubleRow matmul). 512-col bf16 moving (interp) streams at
  full rate with LDW hidden; 128-col DR only partially hides LDW.
- NO bf16 state: the fp8 state is the only state, kept as FOUR quarter
  tiles [P,2,P] so consumers wait per-quarter (whole-TILE dep
  granularity was adding ~1us/boundary). rh also 4 quarter fp8 tiles.
  Elementwise consumers read fp8 (rh, gq, g) — numpy-validated. Hermite
  bases are rebuilt in bf16 from tail intermediates (m1=tq1-gq1,
  d3=tq2-gq2, w3=tq3-gq3, E3=(2*d3+m1)+w3, D=su-g), so no basis
  precision is lost. E3's STT partial runs right after e2 (inputs
  ready) so j9-12 never wait on it.
- Gate PSUM as HALF tiles (psA/psB pools, 8x[P,HALF] = all 8 banks):
  sigmoid/tanh of half 0 start mid-gate instead of after the gate's
  last matmul. tanh+tq+y8 run per-quarter; the next eval's R streams
  kp_i as state quarter i lands (boundary ~1-1.9us, was ~3.2).
- ALL 15 dense-output points are PE scaled-identity accumulation
  groups; for j1-11 the +y0 term is folded into the psum->sbuf copy as
  a DVE tensor_add(psum, y0); j14 copies out on ACT (idle at the tail)
  with an explicit 1.0*y0 identity term. This removed the ~20us
  DVE STT preview pipeline entirely. j1,j2 fill e3's rhb-wait bubble,
  j3,j4 fill e4's; j5-15 stream post-e4 with D (su-g, 2 half tiles)
  as the LAST term of j13-15.
- PSUM accumulation groups must be contiguous AND exclusive (the PE
  mis-accumulates interleaved groups); closed groups BETWEEN groups
  are fine (bubble interp proves it).
- DMA: dummy 4B descriptors first absorb per-queue warm-up (~3us);
  x0p8 leads the SCALAR queue (before the ACT table loads), weights
  lead the SYNC queue in consumption order (wr 256KB+256KB+512KB, wh,
  y0b), wz+identm on scalar. Per-descriptor cost ~1.4us favors few,
  large chunks. Queue q1 sustains ~420GB/s aggregate.
- A 12-matmul warm-up filler (never read, y08-fed) covers the
  y08->weights DMA window; HAM re-throttles the PE to half rate when
  its activity dips (dep-free matmuls float to program start under the
  Tile scheduler — pinning fillers to state deps DELAYS the next eval;
  don't). More/fewer fillers measurably hurt (84-89us vs 81): n=20/28
  DO warm e1/e2 from 126ns to 78ns DR spacing, but the filler's own
  half-clock runtime exceeds the gain (e1 is DMA-paced, so its own
  half-clock run is free; fillers interleaved between e1's jc groups
  also regress — at half clock e1 has no DMA stalls to fill).
- Tail points alternate psA/psB psum pairs: with one pool (4 bufs = 2
  points of lookahead) point N's PE groups wait point N-2's DVE
  copy-add, pacing the PE to the slower add rate. Alternating pools
  (psA is idle post-e4) makes e4+tail one gapless 22.6us PE stream.
- The last three points' copies straddle DVE (j13, j15 adds) and ACT
  (j14 copy) so the final copies don't serialize on one engine.
- HAM re-throttles after EVERY PE idle gap: each eval boundary used to
  run the next eval's R at 126ns/DR for ~25-30 matmuls (~1.6us tax).
  Fix: 10-matmul bridge fillers emitted right after each psU gate_mm,
  with rh8-q0 as stationary — rh8 is ready BEFORE U starts, so the
  scheduler cannot hoist the bridge to program start (dep-free fillers
  float there) and it cannot delay R (state-dep fillers would). The
  bridge executes the instant U's last group issues, holding the
  activity window through the boundary. With n=14 bridges everywhere
  the trace shows NO 126ns blocks after the startup ramp (e2/e3's R
  hold 78ns even in the warm device state where n=10 failed) and the
  last matmul lands at 72.4us. At the fast clock 14x78ns still fits
  inside the ~1.3us boundary gap, so nothing is delayed. The startup
  ramp itself (~8us of half-rate until ~20us) is unfixable: pre-work
  runs at the same half clock it would save.
"""

import numpy as np

import concourse.bacc as bacc
import concourse.bass as bass
import concourse.mybir as mybir
import concourse.tile as tile
from concourse import bass_utils

B, H, T = 1024, 1024, 16
N_CORES = 8
BS = B // N_CORES
P = 128
NK = H // P
NJ = H // P
NK2 = NK // 2
HALF = H // 2
QTR = H // 4

F32 = mybir.dt.float32
BF16 = mybir.dt.bfloat16
FP8 = mybir.dt.float8e4
AF = mybir.ActivationFunctionType
ALU = mybir.AluOpType
DR = mybir.MatmulPerfMode.DoubleRow

# set by the dev harness (test.py) only; grading uses the defaults
TRACE = False
TRACE_DIR = None
LAST_EXEC_NS = None


def _coeffs(t_vals):
    t0, t_end = float(t_vals[0]), float(t_vals[-1])
    h = t_end - t0
    cs = {}
    for j in range(1, T):
        tau = (float(t_vals[j]) - t0) / h
        c01 = 3 * tau**2 - 2 * tau**3
        c10 = (tau**3 - 2 * tau**2 + tau) * h
        c11 = (tau**3 - tau**2) * h
        cj = c01 * h / 6 + c11
        cs[j] = (c01, c10, c11, cj)
    return h, cs


def _plans(t_vals):
    """Per-point PE term plans (no y0 term: it is added in the psum
    copy-out) + deduped identity-coefficient list."""
    h, cs = _coeffs(t_vals)
    plans = {}
    for j in range(1, T):
        c01, c10, c11, cj = cs[j]
        if j <= 2:
            plans[j] = [(2 * (c01 + c11 / h), "d3"), (2 * c10 / h, "m1")]
        elif j <= 8:
            plans[j] = [(c01 + c11 / h, "w3"), (2 * c10 / h, "m1")]
        elif j <= 12:
            plans[j] = [(c01 / 3, "E3"), (2 * c10 / h, "m1"),
                        (cj / h, "w3")]
        else:
            # j13-15: m1's coefficient is tiny near tau=1; dropping it
            # costs ~0.4e-3 total err (proto) and 2 PE terms
            plans[j] = [(c01 / 3, "E3"), (cj, "D")]
    # j14 gets an ACT copy-out instead of a DVE add, so it carries an
    # explicit y0 identity term (j13/j15 use DVE adds: the last three
    # points' copies must straddle ACT and DVE or they serialize at the
    # very end of the kernel)
    plans[14] = [(1.0, "y0")] + plans[14]
    coeffs = []
    index = {}
    for pl in plans.values():
        for c, _ in pl:
            key = float(np.float32(c))
            if key not in index:
                index[key] = len(coeffs)
                coeffs.append(key)
    return h, cs, plans, coeffs, index


def _build_program(t_vals: np.ndarray):
    h, cs, plans, icoeffs, iidx = _plans(t_vals)
    NID = len(icoeffs)

    nc = bacc.Bacc("TRN2", target_bir_lowering=False, debug=False)

    x0pb_d = nc.dram_tensor("x0pb", [P, NK * P], BF16, kind="ExternalInput")
    x0p8_d = nc.dram_tensor("x0p8", [P, NK, P], FP8, kind="ExternalInput")
    # fp8 weights packed [p, jc, kp, i, q] = W[jc*128+q, (2kp+i)*128+p]
    w_d = {nm: nc.dram_tensor(f"w{nm}", [P, NJ, NK2, 2, P], FP8,
                              kind="ExternalInput")
           for nm in ("r", "z", "h")}
    idm_d = nc.dram_tensor("identm", [P, NID * P], BF16,
                           kind="ExternalInput")
    warm_d = nc.dram_tensor("warm", [P, 4], mybir.dt.uint8,
                            kind="ExternalInput")
    out_d = nc.dram_tensor("outp", [T - 1, P, H], BF16,
                           kind="ExternalOutput")

    with tile.TileContext(nc) as tc:
        with (
            tc.tile_pool(name="wpool", bufs=1) as wpool,
            tc.tile_pool(name="state", bufs=1) as state,
            tc.tile_pool(name="work", bufs=1) as work,
            tc.tile_pool(name="psA", bufs=4, space="PSUM") as psA,
            tc.tile_pool(name="psB", bufs=4, space="PSUM") as psB,
        ):
            # --- input DMAs (consumption order) -------------------------
            # x0p8 FIRST (gates the warm-up filler AND eval1), then wr in
            # four 256KB chunks (just-in-time for R1), y0b, wh. wz +
            # identm ride the scalar queue (parallel descriptor setup; Z
            # trails R by ~2us so the ACT-table delay there is harmless).
            # tiny dummy DMAs absorb the per-queue warm-up latency so the
            # real descriptors flow immediately
            wmt = state.tile([P, 4], mybir.dt.uint8, tag="wmt")
            nc.sync.dma_start(wmt[:], warm_d[:, :])
            wmt2 = state.tile([P, 4], mybir.dt.uint8, tag="wmt2")
            nc.scalar.dma_start(wmt2[:], warm_d[:, :])
            # x0p8 leads the SCALAR queue (its trigger precedes the ACT
            # table loads); weights lead the SYNC queue — both queues
            # start filling immediately and e1 is fed ~2us sooner.
            y08 = state.tile([P, NK, P], FP8, tag="y08")
            nc.scalar.dma_start(y08[:], x0p8_d[:, :, :])
            w_sb = {nm: wpool.tile([P, NJ, NK2, 2, P], FP8,
                                   tag=f"w_{nm}", name=f"w_{nm}")
                    for nm in ("r", "z", "h")}
            nc.sync.dma_start(w_sb["r"][:, 0:2], w_d["r"][:, 0:2])
            nc.sync.dma_start(w_sb["r"][:, 2:4], w_d["r"][:, 2:4])
            for jcc in [1]:
                nc.sync.dma_start(w_sb["r"][:, jcc * 4:(jcc + 1) * 4],
                                  w_d["r"][:, jcc * 4:(jcc + 1) * 4])
            for jcc in range(2):
                nc.sync.dma_start(w_sb["h"][:, jcc * 4:(jcc + 1) * 4],
                                  w_d["h"][:, jcc * 4:(jcc + 1) * 4])
                nc.scalar.dma_start(w_sb["z"][:, jcc * 4:(jcc + 1) * 4],
                                    w_d["z"][:, jcc * 4:(jcc + 1) * 4])
            y0b = state.tile([P, H], BF16, tag="y0b")
            nc.sync.dma_start(y0b[:], x0pb_d[:, :])
            idn = wpool.tile([P, NID * P], BF16, tag="idn")
            nc.scalar.dma_start(idn[:], idm_d[:, :])

            def ident(c):
                i = iidx[float(np.float32(c))]
                return idn[:, i * P:(i + 1) * P]

            # --- helpers ------------------------------------------------
            def gate_mm(psh, wt, rhs8, fill=None):
                # fp8 DoubleRow, j-outer: groups contiguous+exclusive.
                # psh: list of 2 half psum tiles [P,HALF] so sigmoid/tanh
                # of half 0 can start mid-gate (whole-tile deps used to
                # stall the activations until the gate's LAST matmul).
                # rhs8: one [P,NK,P] tile or 4 quarter tiles [P,2,P].
                # fill: a psum tile for 4-MM filler groups interleaved
                # between jc groups — they execute inside e1's
                # weight-DMA stalls, keeping the HAM duty (and clock) up
                # so e2 starts at full rate instead of 126ns/DR.
                for jc in range(NJ):
                    dst = psh[jc // 4][:, (jc % 4) * P:(jc % 4 + 1) * P]
                    for kp in range(NK2):
                        mv = (rhs8[kp][:] if isinstance(rhs8, list)
                              else rhs8[:, 2 * kp:2 * kp + 2, :])
                        nc.tensor.matmul(
                            dst,
                            wt[:, jc, kp],
                            mv,
                            start=(kp == 0),
                            stop=(kp == NK2 - 1),
                            perf_mode=DR,
                        )

            def halves(t_):
                return (t_[:, :HALF], t_[:, HALF:])

            def halves8(t_):
                return (t_[:, 0:NK2, :], t_[:, NK2:NK, :])

            def quarters(t_):
                return [t_[:, i * QTR:(i + 1) * QTR] for i in range(4)]

            def emit_filler(tag, n=12, dep=None):
                # HAM keep-warm: never-read matmuls; closed accumulation
                # group. With dep=a state-quarter tile the filler is
                # PINNED to that eval's boundary (the Tile scheduler
                # floats dep-free matmuls to program start).
                ps = psB.tile([P, HALF], F32, tag="psb", name=f"fil{tag}")
                mv = dep if dep is not None else y08
                for i in range(n):
                    rhs = (mv[:] if dep is not None
                           else y08[:, 2 * (i % NK2):2 * (i % NK2) + 2, :])
                    nc.tensor.matmul(
                        ps[:, :P], y08[:, 0:2, :], rhs,
                        start=(i == 0), stop=(i == n - 1),
                        perf_mode=DR,
                    )

            basis = {}

            def bslice(bn, hf):
                b_ = basis[bn]
                if isinstance(b_, list):
                    return b_[hf][:]
                return b_[:, hf * HALF:(hf + 1) * HALF]

            def emit_point(j, copy="dve", pool=None):
                """PE accumulation groups (2 halves) + copy-out + DMA.
                copy='dve': DVE tensor_add folds +y0 into the copy.
                copy='act': plain ACT Copy (plan carries a y0 term).
                pool: psB during evals (psA holds gates); in the tail
                alternate psA/psB for 4 points of lookahead so the PE
                stream is not paced by the DVE copy-adds (psum-buf
                reuse couples point N to point N-2's copy)."""
                pl = plans[j]
                pp_ = pool if pool is not None else psB
                tg = "psa" if pp_ is psA else "psb"
                ph = (pp_.tile([P, HALF], F32, tag=tg,
                               name=f"pi{j}_0"),
                      pp_.tile([P, HALF], F32, tag=tg,
                               name=f"pi{j}_1"))
                for hf in range(2):
                    for i, (c, bn) in enumerate(pl):
                        nc.tensor.matmul(
                            ph[hf][:],
                            ident(c),
                            bslice(bn, hf),
                            start=(i == 0),
                            stop=(i == len(pl) - 1),
                        )
                o = work.tile([P, H], BF16, tag="otile", bufs=4,
                              name=f"o_{j}")
                for hf in range(2):
                    dst = o[:, hf * HALF:(hf + 1) * HALF]
                    if copy == "dve":
                        nc.vector.tensor_add(
                            dst, ph[hf][:],
                            y0b[:, hf * HALF:(hf + 1) * HALF])
                    else:
                        nc.scalar.activation(dst, ph[hf][:], AF.Copy)
                nc.sync.dma_start(out_d[j - 1, :, :], o[:])

            # --- one f-eval + state update ------------------------------
            def q8(y8, i):
                return (y8[i][:] if isinstance(y8, list)
                        else y8[:, 2 * i:2 * i + 2, :])

            def ps_pair(nm):
                return [psA.tile([P, HALF], F32, tag="psa",
                                 name=f"{nm}h{i}") for i in range(2)]

            def eval_stage(name, y8, hscale, last=False, bubble=None,
                           bridge_n=14):
                """Returns (tq, gq, y8_next[list of 4 quarter tiles])."""
                psR = ps_pair(f"psR{name}")
                gate_mm(psR, w_sb["r"], y8)
                psZ = ps_pair(f"psZ{name}")
                gate_mm(psZ, w_sb["z"], y8)

                rb = work.tile([P, H], BF16, tag="rb", bufs=2)
                for d, s in zip(halves(rb), psR):
                    nc.scalar.activation(d, s[:], AF.Sigmoid)
                # rh as 4 quarter tiles: U's kp_i waits quarter i only
                rh8 = [work.tile([P, 2, P], FP8, tag=f"rh8q{i}", bufs=2,
                                 name=f"rh8{name}q{i}") for i in range(4)]
                rbq = quarters(rb)
                for i in range(4):
                    nc.vector.tensor_mul(rh8[i][:], rbq[i], q8(y8, i))

                sneg = work.tile([P, H], BF16, tag="sneg", bufs=2,
                                 name=f"sneg{name}")
                for d, s in zip(halves(sneg), psZ):
                    nc.scalar.activation(d, s[:], AF.Sigmoid, scale=-1.0)
                if hscale != 1.0:
                    sc = work.tile([P, H], BF16, tag="sc", bufs=2,
                                   name=f"sc{name}")
                    nc.vector.tensor_scalar_mul(sc[:], sneg[:],
                                                float(hscale))
                else:
                    sc = sneg
                scq = quarters(sc)

                if last:
                    # g = sneg*y4; su/D in the tail
                    gb = work.tile([P, H], BF16, tag="gb")
                    gbq = quarters(gb)
                    for i in range(4):
                        nc.vector.tensor_mul(gbq[i], scq[i], q8(y8, i))
                    if bubble is not None:
                        bubble()
                    psU = ps_pair(f"psU{name}")
                    gate_mm(psU, w_sb["h"], rh8)
                    ub = work.tile([P, H], BF16, tag="ub", bufs=2,
                                   name=f"u{name}")
                    sub = work.tile([P, H], BF16, tag="sub")
                    uq, suq = quarters(ub), quarters(sub)
                    # D as two half TILES so j13's half-0 group isn't
                    # gated on half 1 (whole-tile dep granularity)
                    Dh = [work.tile([P, HALF], BF16, tag=f"Dh{i}",
                                    name=f"Dh{i}") for i in range(2)]
                    for i in range(4):
                        nc.scalar.activation(
                            uq[i], psU[i // 2][:, (i % 2) * QTR:
                                               (i % 2 + 1) * QTR],
                            AF.Tanh)
                        nc.vector.tensor_mul(suq[i], scq[i], uq[i])
                        if i % 2 == 1:
                            nc.vector.tensor_sub(
                                Dh[i // 2][:],
                                sub[:, (i // 2) * HALF:(i // 2 + 1) * HALF],
                                gb[:, (i // 2) * HALF:(i // 2 + 1) * HALF])
                    return Dh

                # per-quarter prelude: gq_i, q_i cleared off DVE before
                # the tail needs it
                gq = work.tile([P, H], BF16, tag="gq", bufs=2,
                               name=f"gq{name}")
                gqq = quarters(gq)
                q = work.tile([P, H], BF16, tag="q", bufs=2,
                              name=f"q{name}")
                qq = quarters(q)
                y0q = quarters(y0b)
                for i in range(4):
                    nc.vector.tensor_mul(gqq[i], scq[i], q8(y8, i))
                    nc.vector.tensor_sub(qq[i], y0q[i], gqq[i])

                if bubble is not None:
                    bubble()

                psU = ps_pair(f"psU{name}")
                gate_mm(psU, w_sb["h"], rh8)
                # boundary bridge: filler matmuls whose dep (rh8 q0) is
                # ready BEFORE U starts, so they execute the instant U's
                # last group issues — the HAM activity window never dips
                # across the boundary (it re-throttles the next eval's R
                # to 126ns/DR for ~3us after any PE idle gap). Dep-free
                # fillers get hoisted to program start by the scheduler;
                # state-dep fillers would delay R. rh8 is the one tile
                # that pins them here harmlessly.
                fb = psB.tile([P, HALF], F32, tag="psb",
                              name=f"fbr{name}")
                for i in range(bridge_n):
                    nc.tensor.matmul(
                        fb[:, :P], rh8[0][:],
                        y08[:, 2 * (i % NK2):2 * (i % NK2) + 2, :],
                        start=(i == 0), stop=(i == bridge_n - 1),
                        perf_mode=DR,
                    )
                ub = work.tile([P, H], BF16, tag="ub", bufs=2,
                               name=f"u{name}")
                tq = work.tile([P, H], BF16, tag="tq", bufs=2,
                               name=f"tq{name}")
                # state as 4 quarter tiles: next eval's kp_i dep is
                # quarter i's add, not the whole state
                y8n = [state.tile([P, 2, P], FP8, tag=f"y8{name}q{i}",
                                  name=f"y8{name}q{i}")
                       for i in range(4)]
                uq, tqq = quarters(ub), quarters(tq)
                for i in range(4):
                    nc.scalar.activation(
                        uq[i], psU[i // 2][:, (i % 2) * QTR:
                                           (i % 2 + 1) * QTR],
                        AF.Tanh)
                    nc.vector.tensor_mul(tqq[i], scq[i], uq[i])
                    nc.vector.tensor_add(y8n[i][:], qq[i], tqq[i])
                return tq, gq, y8n

            # --- integration --------------------------------------------
            emit_filler("w0", n=12)
            tq1, gq1, y28 = eval_stage("e1", y08, h / 2, bridge_n=14)
            m1b = work.tile([P, H], BF16, tag="m1b")
            nc.vector.tensor_sub(m1b[:], tq1[:], gq1[:])
            basis["m1"] = m1b
            

            tq2, gq2, y38 = eval_stage("e2", y28, h / 2)
            d3b = work.tile([P, H], BF16, tag="d3b")
            nc.vector.tensor_sub(d3b[:], tq2[:], gq2[:])
            basis["d3"] = d3b
            # E3 partial (2*d3 + m1) early: its inputs exist now, and
            # j9-12's E3 terms fire right after e4's gates
            E3a = work.tile([P, H], BF16, tag="E3a")
            nc.vector.scalar_tensor_tensor(
                E3a[:], d3b[:], 2.0, m1b[:], ALU.mult, ALU.add)
            

            def bubble3():
                emit_point(1)
                emit_point(2)

            tq3, gq3, y48 = eval_stage("e3", y38, h, bubble=bubble3)
            w3b = work.tile([P, H], BF16, tag="w3b")
            nc.vector.tensor_sub(w3b[:], tq3[:], gq3[:])
            basis["w3"] = w3b
            E3b = work.tile([P, H], BF16, tag="E3b")
            nc.vector.tensor_add(E3b[:], E3a[:], w3b[:])
            basis["E3"] = E3b

            

            def bubble4():
                emit_point(3)
                emit_point(4)

            Dh = eval_stage("e4", y48, 1.0, last=True, bubble=bubble4)
            basis["D"] = Dh
            basis["y0"] = y0b

            # --- remaining dense-output points (PE) ---------------------
            # copies split DVE/ACT so neither engine serializes the tail;
            # j9 is a DVE STT preview emitted LAST so its STTs fill the
            # DVE tail idle without blocking the copy-adds
            def emit_j9():
                c01, c10, c11, _ = cs[9]
                o1 = work.tile([P, H], BF16, tag="o1", name="o1_9")
                nc.vector.scalar_tensor_tensor(
                    o1[:], w3b[:], float(c01 + c11 / h), y0b[:],
                    ALU.mult, ALU.add)
                o9 = work.tile([P, H], BF16, tag="otile", bufs=4,
                               name="o_9")
                nc.vector.scalar_tensor_tensor(
                    o9[:], m1b[:], float(2 * c10 / h), o1[:],
                    ALU.mult, ALU.add)
                nc.sync.dma_start(out_d[8, :, :], o9[:])

            for n_, j in enumerate([5, 6, 7, 8, 10, 11, 12, 13, 14, 15]):
                emit_point(j, copy=("act" if j == 14 else "dve"),
                           pool=(psA if n_ % 2 else psB))
                if j == 8:
                    emit_j9()

    nc.compile()
    return nc


def kernel(x0, t, W_hr, W_hz, W_hh):
    import ml_dtypes
    bf = ml_dtypes.bfloat16
    f8 = ml_dtypes.float8_e4m3
    x0 = np.ascontiguousarray(np.asarray(x0, dtype=np.float32))
    t = np.asarray(t, dtype=np.float32)

    def pack_w8(W):
        # w8[p, jc, kp, i, q] = W[jc*128+q, (2kp+i)*128+p]
        wt = np.asarray(W, dtype=np.float32).T.reshape(
            NK2, 2, P, NJ, P)
        return np.ascontiguousarray(
            wt.transpose(2, 3, 0, 1, 4).astype(f8))

    wr_p, wz_p, wh_p = pack_w8(W_hr), pack_w8(W_hz), pack_w8(W_hh)

    _, _, _, icoeffs, _ = _plans(t)
    eye = np.eye(P, dtype=np.float32)
    identm = np.ascontiguousarray(
        np.stack([c * eye for c in icoeffs])    # [NID, P, P]
        .transpose(1, 0, 2).reshape(P, -1).astype(bf))

    nc = _build_program(t)

    in_maps = []
    for c in range(N_CORES):
        xc = x0[c * BS:(c + 1) * BS]
        xp = np.ascontiguousarray(
            xc.T.reshape(NK, P, BS).transpose(1, 0, 2)).reshape(P, NK * BS)
        in_maps.append({
            "x0pb": np.ascontiguousarray(xp.astype(bf)),
            "x0p8": np.ascontiguousarray(
                xp.astype(f8).reshape(P, NK, BS)),
            "wr": wr_p, "wz": wz_p, "wh": wh_p,
            "identm": identm,
        })
    kw = {}
    if TRACE:
        kw = dict(trace=True, tmpdir=TRACE_DIR)
    res = bass_utils.run_bass_kernel_spmd(
        nc, in_maps, core_ids=list(range(N_CORES)), **kw)
    global LAST_EXEC_NS
    LAST_EXEC_NS = res.exec_time_ns

    full = np.empty((B, T, H), dtype=np.float32)
    full[:, 0, :] = x0
    for c in range(N_CORES):
        op = np.asarray(res.results[c]["outp"]).astype(np.float32)
        op = op.reshape(T - 1, P, NK, BS).transpose(3, 0, 2, 1)
        full[c * BS:(c + 1) * BS, 1:, :] = np.ascontiguousarray(
            op).reshape(BS, T - 1, H)
    return full
